# revision 7
# baseline (speedup 1.0000x reference)
"""Trainium2 Bass kernel for nn_CostTokenizer.

Computes, for 3 pyramid levels: local correlation (radius 3 -> 49 circular
shifts) between consecutive frame pairs, then a 1x1 conv projection K=49 ->
D=192 (+bias).  Data-parallel over the 8 (pair, batch) units across the 8
NeuronCores; each core handles one (p, b) for all three levels.

Per-core algorithm (per level, channels C on SBUF partitions):
  - host pre-pads fb circularly ([C, H+6, Weven]) in fp16, and ships a
    partition-duplicated copy so one 128-wide DVE tensor_tensor multiply
    computes the shifted products of TWO displacement shifts at once
    (bottom 64 partitions hold fb pre-shifted by 2 columns).
  - TensorE reduces products over channels with block-ones stationary
    matrices, accumulating all 49 correlation rows (+ a constant ones row
    for the bias) into PSUM [50, 512] tiles.
  - TensorE applies the 1x1 conv as [50, D] matmuls over the corr rows
    (weights pre-permuted/scaled on host, bias folded in via the ones row).
"""

import math
import sys

import numpy as np

sys.path.insert(0, "/opt/trn_rl_repo")

RADIUS = 3
D = 192
K = 49
NCORES = 8

# (C, H, W) per level
LEVELS = [
    dict(C=64, H=128, W=128, STRIP=2048),
    dict(C=128, H=64, W=64, STRIP=2048),
    dict(C=192, H=32, W=32, STRIP=1024),
]


def _geom(lv):
    H, W = lv["H"], lv["W"]
    PR = H + 6
    PW = W + 6 + ((W + 6) % 2)  # even padded row width
    return PR, PW


def level_passes(level_idx):
    """Pass table for one level.

    Returns list of passes. Each pass is a dict:
      P        : partition count of the products op (128 or 64)
      fb       : which fb buffer: 'e'/'o' (main) or 're'/'ro' (remainder, L3)
      fa       : 'main' or 'rem'
      dy, dxa  : shift of the TOP half (bottom half, if P=128-pair, is dxa+2)
      rows     : tuple of corr row slots written (1 or 2)
      col_off  : element offset into the padded row = (3 - dxa) - odd_adjust
    Also returns slot_map: row slot -> (dy, dx).
    """
    passes = []
    slot_map = {}
    C = LEVELS[level_idx]["C"]

    def col_parity(dxa):
        return (3 - dxa) % 2  # 1 -> use odd buffer at offset-1

    if level_idx == 0:
        # C=64: everything is dup-paired over (dxa, dxa+2), singles for dx=2
        p = 0
        for dy in range(-3, 4):
            for dxa in (-3, -2, 1):
                r0 = 2 * (3 * (dy + 3) + {-3: 0, -2: 1, 1: 2}[dxa])
                odd = col_parity(dxa)
                passes.append(dict(P=128, fb=("o" if odd else "e"), fa="main",
                                   dy=dy, dxa=dxa, rows=(r0, r0 + 1),
                                   col_off=(3 - dxa) - odd))
                slot_map[r0] = (dy, dxa)
                slot_map[r0 + 1] = (dy, dxa + 2)
        for dy in range(-3, 4):
            r = 42 + (dy + 3)
            odd = col_parity(2)
            passes.append(dict(P=64, fb=("o" if odd else "e"), fa="main",
                               dy=dy, dxa=2, rows=(r,),
                               col_off=(3 - 2) - odd))
            slot_map[r] = (dy, 2)
    elif level_idx == 1:
        # C=128: single full-width pass per shift
        for dy in range(-3, 4):
            for dx in range(-3, 4):
                r = 7 * (dy + 3) + (dx + 3)
                odd = col_parity(dx)
                passes.append(dict(P=128, fb=("o" if odd else "e"), fa="main",
                                   dy=dy, dxa=dx, rows=(r,),
                                   col_off=(3 - dx) - odd))
                slot_map[r] = (dy, dx)
    else:
        # C=192: 49 full-width passes on ch 0:128, plus dup-paired remainder
        # passes on ch 128:192.  Row order per dy chosen so remainder pairs
        # land on adjacent slots.
        order = [-3, -1, -2, 0, 1, 3, 2]
        for dy in range(-3, 4):
            for dx in range(-3, 4):
                r = 7 * (dy + 3) + order.index(dx)
                slot_map[r] = (dy, dx)
        for dy in range(-3, 4):
            for dx in range(-3, 4):
                r = 7 * (dy + 3) + order.index(dx)
                odd = col_parity(dx)
                passes.append(dict(P=128, fb=("o" if odd else "e"), fa="main",
                                   dy=dy, dxa=dx, rows=(r,),
                                   col_off=(3 - dx) - odd))
        for dy in range(-3, 4):
            for dxa in (-3, -2, 1):
                r0 = 7 * (dy + 3) + order.index(dxa)
                assert order.index(dxa) + 1 == order.index(dxa + 2)
                odd = col_parity(dxa)
                passes.append(dict(P=128, fb=("ro" if odd else "re"), fa="rem",
                                   dy=dy, dxa=dxa, rows=(r0, r0 + 1),
                                   col_off=(3 - dxa) - odd))
        for dy in range(-3, 4):
            r = 7 * (dy + 3) + order.index(2)
            odd = col_parity(2)
            passes.append(dict(P=64, fb=("ro" if odd else "re"), fa="rem",
                               dy=dy, dxa=2, rows=(r,),
                               col_off=(3 - 2) - odd))
    return passes, slot_map


# ---------------------------------------------------------------------------
# host-side data prep
# ---------------------------------------------------------------------------

def _pad_wrap(fb):
    """fb [C, H, W] fp16 -> [C, H+6, PW] with circular 3-halo, PW even."""
    C, H, W = fb.shape
    PR, PW = H + 6, W + 6 + ((W + 6) % 2)
    out = np.zeros((C, PR, PW), dtype=np.float16)
    padded = np.pad(fb, ((0, 0), (3, 3), (3, 3)), mode="wrap")
    out[:, :, : W + 6] = padded
    return out


def _dup_pair(pad_top, pad_bot_src):
    """Stack [64, PR, PW] on top of the same data shifted +2 columns."""
    C, PR, PW = pad_top.shape
    assert C == 64 and pad_bot_src.shape == pad_top.shape
    bot = np.zeros_like(pad_bot_src)
    bot[:, :, 2:] = pad_bot_src[:, :, :-2]
    return np.concatenate([pad_top, bot], axis=0).reshape(128, PR * PW)


def _odd_shift(flat):
    """buf_odd[x] = buf_even[x+1] (flat element shift)."""
    out = np.zeros_like(flat)
    out[:, :-1] = flat[:, 1:]
    return out


def make_core_inputs(fa1, fb1, fa2, fb2, fa3, fb3, w1, b1, w2, b2, w3, b3):
    """All f* are fp32 [C, H, W] for one (pair, batch). Returns in_map dict."""
    ins = {}
    fs = [(fa1.astype(np.float16), fb1.astype(np.float16)),
          (fa2.astype(np.float16), fb2.astype(np.float16)),
          (fa3.astype(np.float16), fb3.astype(np.float16))]
    ws = [(w1, b1), (w2, b2), (w3, b3)]

    # level 1: fa duplicated; fb padded + dup-paired, even & odd variants
    fa, fb = fs[0]
    C, H, W = fa.shape
    faf = fa.reshape(C, H * W)
    ins["fa1"] = np.concatenate([faf, faf], axis=0)
    pad = _pad_wrap(fb)
    fb_e = _dup_pair(pad, pad)
    ins["fb1e"] = fb_e
    ins["fb1o"] = _odd_shift(fb_e)

    # level 2: full-width, no dup
    fa, fb = fs[1]
    C, H, W = fa.shape
    ins["fa2"] = fa.reshape(C, H * W)
    pad = _pad_wrap(fb)
    fb_e = pad.reshape(C, -1)
    ins["fb2e"] = fb_e
    ins["fb2o"] = _odd_shift(fb_e)

    # level 3: main (ch 0:128) + remainder (ch 128:192 dup-paired)
    fa, fb = fs[2]
    C, H, W = fa.shape
    ins["fa3"] = fa[:128].reshape(128, H * W)
    far = fa[128:].reshape(64, H * W)
    ins["fa3r"] = np.concatenate([far, far], axis=0)
    pad_m = _pad_wrap(fb[:128])
    fb_e = pad_m.reshape(128, -1)
    ins["fb3e"] = fb_e
    ins["fb3o"] = _odd_shift(fb_e)
    pad_r = _pad_wrap(fb[128:])
    fbr_e = _dup_pair(pad_r, pad_r)
    ins["fb3re"] = fbr_e
    ins["fb3ro"] = _odd_shift(fbr_e)

    # conv weights: [50, 192], rows permuted to corr slot order, scaled by
    # 1/sqrt(C); row 49 = bias (matched with the constant-1 corr row)
    for li, (w, b) in enumerate(ws):
        _, slot_map = level_passes(li)
        Cl = LEVELS[li]["C"]
        wb = np.zeros((50, D), dtype=np.float16)
        for slot, (dy, dx) in slot_map.items():
            kflat = (dy + 3) * 7 + (dx + 3)
            wb[slot] = (w[:, kflat] / math.sqrt(Cl)).astype(np.float16)
        wb[49] = b.astype(np.float16)
        ins[f"wb{li + 1}"] = wb

    # ones patterns for the reduce matmuls (shared across levels):
    #   ones_full [128, 99]: col 49 all-ones     -> view [:, 49-j:99-j]
    #   ones_pair [128,100]: col 49 ones on top half, col 50 on bottom half
    of = np.zeros((128, 99), dtype=np.float16)
    of[:, 49] = 1.0
    op = np.zeros((128, 100), dtype=np.float16)
    op[:64, 49] = 1.0
    op[64:, 50] = 1.0
    ins["ones_full"] = of
    ins["ones_pair"] = op
    return ins


# ---------------------------------------------------------------------------
# bass kernel
# ---------------------------------------------------------------------------

_CACHED = {}


def build_bass():
    import concourse.bass as bass
    import concourse.tile as tile
    from concourse import bacc, mybir

    fp16 = mybir.dt.float16
    fp32 = mybir.dt.float32

    nc = bacc.Bacc("TRN2", target_bir_lowering=False, debug=False,
                   enable_asserts=False, num_devices=NCORES)

    # --- declare I/O ---
    dram = {}

    def din(name, shape, dt=fp16):
        dram[name] = nc.dram_tensor(name, list(shape), dt,
                                    kind="ExternalInput").ap()

    PR1, PW1 = _geom(LEVELS[0])
    PR2, PW2 = _geom(LEVELS[1])
    PR3, PW3 = _geom(LEVELS[2])
    din("fa1", (128, 128 * 128))
    din("fb1e", (128, PR1 * PW1))
    din("fb1o", (128, PR1 * PW1))
    din("fa2", (128, 64 * 64))
    din("fb2e", (128, PR2 * PW2))
    din("fb2o", (128, PR2 * PW2))
    din("fa3", (128, 32 * 32))
    din("fa3r", (128, 32 * 32))
    din("fb3e", (128, PR3 * PW3))
    din("fb3o", (128, PR3 * PW3))
    din("fb3re", (128, PR3 * PW3))
    din("fb3ro", (128, PR3 * PW3))
    din("wb1", (50, D))
    din("wb2", (50, D))
    din("wb3", (50, D))
    din("ones_full", (128, 99))
    din("ones_pair", (128, 100))

    outs = {}
    for li, lv in enumerate(LEVELS):
        outs[li] = nc.dram_tensor(f"t{li + 1}", [D, lv["H"] * lv["W"]], fp32,
                                  kind="ExternalOutput").ap()

    mult = mybir.AluOpType.mult

    with tile.TileContext(nc) as tc:
        with tc.tile_pool(name="const", bufs=1) as cpool:
            ones_full = cpool.tile([128, 99], fp16)
            nc.sync.dma_start(ones_full[:], dram["ones_full"][:])
            ones_pair = cpool.tile([128, 100], fp16)
            nc.sync.dma_start(ones_pair[:], dram["ones_pair"][:])
            wbs = {}
            for li in range(3):
                wbs[li] = cpool.tile([50, D], fp16, tag=f"wb{li}", name=f"wb{li}")
                nc.sync.dma_start(wbs[li][:], dram[f"wb{li + 1}"][:])

            for li, lv in enumerate(LEVELS):
                C, H, W = lv["C"], lv["H"], lv["W"]
                PR, PW = _geom(lv)
                N = H * W
                STRIP = lv["STRIP"]
                RS = STRIP // W          # image rows per strip
                n_strips = N // STRIP
                n_sub = STRIP // 512
                passes, _ = level_passes(li)
                npass = len(passes)

                with tc.tile_pool(name=f"L{li}_in", bufs=1) as inpool, \
                     tc.tile_pool(name=f"L{li}_prod", bufs=4) as prodpool, \
                     tc.tile_pool(name=f"L{li}_corr", bufs=1) as corrpool, \
                     tc.tile_pool(name=f"L{li}_ps", bufs=4, space="PSUM") as pspool, \
                     tc.tile_pool(name=f"L{li}_psc", bufs=2, space="PSUM") as pscpool, \
                     tc.tile_pool(name=f"L{li}_tok", bufs=3) as tokpool:

                    # load this level's tensors
                    sb = {}
                    if li == 0:
                        names = ["fa1", "fb1e", "fb1o"]
                    elif li == 1:
                        names = ["fa2", "fb2e", "fb2o"]
                    else:
                        names = ["fa3", "fa3r", "fb3e", "fb3o", "fb3re",
                                 "fb3ro"]
                    for nm in names:
                        t = inpool.tile(list(dram[nm].shape), fp16, tag=nm)
                        nc.sync.dma_start(t[:], dram[nm][:])
                        sb[nm] = t

                    if li == 0:
                        fa_of = {"main": sb["fa1"]}
                        fb_of = {"e": sb["fb1e"], "o": sb["fb1o"]}
                    elif li == 1:
                        fa_of = {"main": sb["fa2"]}
                        fb_of = {"e": sb["fb2e"], "o": sb["fb2o"]}
                    else:
                        fa_of = {"main": sb["fa3"], "rem": sb["fa3r"]}
                        fb_of = {"e": sb["fb3e"], "o": sb["fb3o"],
                                 "re": sb["fb3re"], "ro": sb["fb3ro"]}

                    # two persistent corr buffers (ping-pong), ones row preset
                    corr_bufs = []
                    for i in range(2):
                        cb = corrpool.tile([50, STRIP], fp16, tag=f"corr{i}")
                        nc.vector.memset(cb[:, :], 1.0)
                        corr_bufs.append(cb)

                    for s in range(n_strips):
                        corr_sb = corr_bufs[s % 2]
                        ps = [pspool.tile([50, 512], fp32, space="PSUM", tag="cps", name="cps")
                              for _ in range(n_sub)]
                        for pi, pa in enumerate(passes):
                            P = pa["P"]
                            fa_t = fa_of[pa["fa"]]
                            fb_t = fb_of[pa["fb"]]
                            off = (RS * s + 3 - pa["dy"]) * PW + pa["col_off"]
                            fb_v = _view_rows(off, PW, RS, W, fb_t, P)
                            fa_v = fa_t[0:P, STRIP * s:STRIP * (s + 1)] \
                                .rearrange("p (r x) -> p r x", x=W)
                            prod = prodpool.tile([P, STRIP], fp16, tag="prod")
                            prod_v = prod[0:P, :].rearrange(
                                "p (r x) -> p r x", x=W)
                            nc.vector.tensor_tensor(
                                out=prod_v, in0=fa_v, in1=fb_v, op=mult)

                            # reduce over channels into corr psum rows
                            r0 = pa["rows"][0]
                            if len(pa["rows"]) == 2:
                                lhsT = ones_pair[0:P, 49 - r0:99 - r0]
                            else:
                                lhsT = ones_full[0:P, 49 - r0:99 - r0]
                            for c in range(n_sub):
                                nc.tensor.matmul(
                                    out=ps[c][:, :],
                                    lhsT=lhsT,
                                    rhs=prod[0:P, 512 * c:512 * (c + 1)],
                                    start=(pi == 0), stop=(pi == npass - 1))

                        for c in range(n_sub):
                            nc.scalar.copy(
                                out=corr_sb[0:49, 512 * c:512 * (c + 1)],
                                in_=ps[c][0:49, :])

                        # 1x1 conv: tok[d, n] = wb.T @ corr50
                        tok_hi = tokpool.tile([128, STRIP], fp32, tag="tokhi")
                        tok_lo = tokpool.tile([64, STRIP], fp32, tag="toklo")
                        for c in range(n_sub):
                            pt = pscpool.tile([128, 512], fp32, space="PSUM",
                                              tag="ptc")
                            nc.tensor.matmul(
                                out=pt[:, :], lhsT=wbs[li][:, 0:128],
                                rhs=corr_sb[:, 512 * c:512 * (c + 1)],
                                start=True, stop=True)
                            nc.scalar.copy(
                                out=tok_hi[:, 512 * c:512 * (c + 1)],
                                in_=pt[:, :])
                            pt2 = pscpool.tile([64, 512], fp32, space="PSUM",
                                               tag="ptc2")
                            nc.tensor.matmul(
                                out=pt2[:, :], lhsT=wbs[li][:, 128:192],
                                rhs=corr_sb[:, 512 * c:512 * (c + 1)],
                                start=True, stop=True)
                            nc.scalar.copy(
                                out=tok_lo[:, 512 * c:512 * (c + 1)],
                                in_=pt2[:, :])
                        nc.sync.dma_start(
                            out=outs[li][0:128, STRIP * s:STRIP * (s + 1)],
                            in_=tok_hi[:, :])
                        nc.sync.dma_start(
                            out=outs[li][128:192, STRIP * s:STRIP * (s + 1)],
                            in_=tok_lo[:, :])

    nc.compile()
    return nc


def _view_rows(off, PW, RS, W, fb_t, P):
    """[P, RS, W] view of flat fb tile starting at element `off`."""
    r0, c0 = divmod(off, PW)
    assert c0 + W <= PW, "window crosses padded row end"
    return fb_t[0:P, :].rearrange("p (r x) -> p r x", x=PW)[
        :, r0:r0 + RS, c0:c0 + W]


def kernel(f_lvl1, f_lvl2, f_lvl3, w1, b1, w2, b2, w3, b3):
    from concourse.bass_utils import run_bass_kernel_spmd

    if "nc" not in _CACHED:
        _CACHED["nc"] = build_bass()
    nc = _CACHED["nc"]

    f_lvl1 = np.asarray(f_lvl1)
    f_lvl2 = np.asarray(f_lvl2)
    f_lvl3 = np.asarray(f_lvl3)
    w1, b1 = np.asarray(w1), np.asarray(b1)
    w2, b2 = np.asarray(w2), np.asarray(b2)
    w3, b3 = np.asarray(w3), np.asarray(b3)

    in_maps = []
    for p in range(4):
        for b in range(2):
            in_maps.append(make_core_inputs(
                f_lvl1[p, b], f_lvl1[p + 1, b],
                f_lvl2[p, b], f_lvl2[p + 1, b],
                f_lvl3[p, b], f_lvl3[p + 1, b],
                w1, b1, w2, b2, w3, b3))

    _CACHED["in_maps"] = in_maps
    res = run_bass_kernel_spmd(nc, in_maps, list(range(NCORES)))
    results = res.results

    t_all = []
    for li, lv in enumerate(LEVELS):
        H, W = lv["H"], lv["W"]
        t = np.zeros((4, 2, D, H, W), dtype=np.float32)
        for p in range(4):
            for b in range(2):
                t[p, b] = results[p * 2 + b][f"t{li + 1}"].reshape(D, H, W)
        t_all.append(t)
    return tuple(t_all)
